# revision 1
# baseline (speedup 1.0000x reference)
"""CapsuleLayer (dynamic routing) Trainium2 kernel.

Full inputs -> batch-sharded over 8 NeuronCores -> full output.

Math (per sample b):
    ihat[i,c,o] = sum_d x[i,d] * W[i,c,d,o]
    bias = 0
    for r in 0..2:
        coup = softmax(bias, axis=c)
        s[c,o] = sum_i coup[i,c] * ihat[i,c,o]
        v = squash(s)
        if r < 2: bias[i,c] += sum_o ihat[i,c,o] * v[c,o]
    return v

Device layout (per core, 32 local samples, batch-tiles of 8):
    SBUF partition dim p = (b, i_sub): p = b*16 + i_sub   (b in 0..7 of tile,
    i_sub = i % 16), free dim (ig, c, o) with ig = i // 16 (72 groups).
    ihat tile: [128, 72*10*16]

    einsum: per (bt, ig) one matmul, lhsT = host-prepared block-diagonal
    x tile [ (i_sub,d)=128, (b,i_sub')=128 ], rhs = W chunk [128, 160].

    routing weighted sum: per ig matmul with lhsT = coupZ [128, (c',b')=80]
    (coup placed on the b'=b diagonal via a mask multiply), PSUM-accumulated
    over ig. The [80,160] result holds s[b,c,:] in its (c'==c) diagonal
    blocks; it is masked on evacuation, squashed with per-partition scalars,
    and collapsed to [8,160] with a selector matmul (engine partition ranges
    must start 32-aligned, so no sub-32 partition slicing anywhere).
"""

import sys

if "/opt/trn_rl_repo" not in sys.path:
    sys.path.insert(0, "/opt/trn_rl_repo")

import numpy as np

B, I, D, C, O = 256, 1152, 8, 10, 16
NCORES = 8
BL = B // NCORES            # 32 local samples per core
NBT, BT = 4, 8              # batch tiles
ISUB = 16                   # i's per group
IG = I // ISUB              # 72 groups
CO = C * O                  # 160
NR = 3
EPS = 1e-7
XZ_CHUNK = 18               # ig's per xz DMA chunk
F32 = np.float32

# bf16 for PE-heavy tensors (einsum inputs, ihat storage, coupling weights);
# routing state (bias, coup, softmax, squash, v) stays fp32.
USE_BF16 = True

_compiled = {}


def _build_program():
    import concourse.bacc as bacc
    import concourse.tile as tile
    import concourse.mybir as mybir
    import concourse.bass as bass

    f32 = mybir.dt.float32
    lo = mybir.dt.bfloat16 if USE_BF16 else f32
    nc = bacc.Bacc("TRN2", target_bir_lowering=False, debug=False,
                   num_devices=NCORES)

    xz_t = nc.dram_tensor("xz", [NBT * IG, 128, 128], lo, kind="ExternalInput")
    xt_t = nc.dram_tensor("xt", [128, IG, BL], lo, kind="ExternalInput")
    w_t = nc.dram_tensor("w", [128, IG * CO], lo, kind="ExternalInput")
    cmask_t = nc.dram_tensor("cmask", [C * BT, CO], f32, kind="ExternalInput")
    maskz_t = nc.dram_tensor("maskz", [128, C * BT], lo, kind="ExternalInput")
    sel_t = nc.dram_tensor("sel", [C * BT, BT], f32, kind="ExternalInput")
    out_t = nc.dram_tensor("out", [BL, CO], f32, kind="ExternalOutput")
    vscr_t = nc.dram_tensor("vscr", [BL, CO], f32)   # internal scratch
    xz_ap, xt_ap, w_ap = xz_t.ap(), xt_t.ap(), w_t.ap()
    out_ap, vscr_ap = out_t.ap(), vscr_t.ap()

    AF = mybir.ActivationFunctionType
    ALU = mybir.AluOpType
    AX = mybir.AxisListType

    with tile.TileContext(nc) as tc:
        from contextlib import ExitStack

        with ExitStack() as ctx:
            singles = ctx.enter_context(tc.tile_pool(name="singles", bufs=1))
            xzp = ctx.enter_context(tc.tile_pool(name="xzp", bufs=3))
            psum = ctx.enter_context(tc.tile_pool(name="psum", bufs=4, space="PSUM"))
            psm = ctx.enter_context(tc.tile_pool(name="psm", bufs=1, space="PSUM"))
            tch = ctx.enter_context(tc.tile_pool(name="tch", bufs=2))
            sm = ctx.enter_context(tc.tile_pool(name="sm", bufs=2))

            w_sb = singles.tile([128, IG * CO], lo)
            nc.sync.dma_start(out=w_sb, in_=w_ap)
            xt_sb = singles.tile([128, IG * BL], lo)
            nc.sync.dma_start(out=xt_sb,
                              in_=xt_ap.rearrange("p g b -> p (g b)"))
            cmask = singles.tile([C * BT, CO], f32)
            nc.sync.dma_start(out=cmask, in_=cmask_t.ap())
            maskz = singles.tile([128, C * BT], lo)
            nc.sync.dma_start(out=maskz, in_=maskz_t.ap())
            sel_sb = singles.tile([C * BT, BT], f32)
            nc.sync.dma_start(out=sel_sb, in_=sel_t.ap())

            ihp = ctx.enter_context(tc.tile_pool(name="ihp", bufs=2))
            zsc = singles.tile([128, IG * C * BT], lo)      # coupZ
            bias = singles.tile([128, IG * C], f32)
            tmp720 = singles.tile([128, IG * C], f32)
            coup = singles.tile([128, IG * C], f32)
            zsum = singles.tile([128, IG], f32)
            vrep = singles.tile([128, CO], lo)

            # ---- r0 weighted sum: s0 = 0.1 * sum_{i,d} x*W  (all 32 b) ----
            ps0 = psm.tile([BL, CO], f32)
            for kc in range(IG):
                nc.tensor.matmul(ps0, xt_sb[:, kc * BL:(kc + 1) * BL],
                                 w_sb[:, kc * CO:(kc + 1) * CO],
                                 start=(kc == 0), stop=(kc == IG - 1))
            s_all = singles.tile([BL, CO], f32)
            nc.scalar.mul(s_all, ps0, 1.0 / C)

            # ---- squash32: reference squash on a [32, (c,o)] tile --------
            def nr_rsqrt(pool, a, p, w):
                """exact-ish rsqrt(a) via Sqrt table seed + 2 Newton steps"""
                sq = pool.tile([p, w], f32)
                nc.scalar.activation(sq, a, AF.Sqrt)
                rs = pool.tile([p, w], f32)
                nc.vector.reciprocal(rs, sq)
                t1 = pool.tile([p, w], f32)
                t2 = pool.tile([p, w], f32)
                for _ in range(2):
                    nc.vector.tensor_mul(t1, a, rs)
                    nc.vector.tensor_mul(t1, t1, rs)
                    nc.vector.tensor_scalar(t2, t1, -0.5, 1.5,
                                            op0=ALU.mult, op1=ALU.add)
                    nc.vector.tensor_mul(rs, rs, t2)
                return rs

            def squash_factor(pool, n2, p, w):
                """f = n2 / ((1+n2) * sqrt(n2+eps)), elementwise [p, w]"""
                a = pool.tile([p, w], f32)
                nc.vector.tensor_scalar_add(a, n2, EPS)
                rs = nr_rsqrt(pool, a, p, w)
                dn = pool.tile([p, w], f32)
                nc.vector.tensor_scalar_add(dn, n2, 1.0)
                di = pool.tile([p, w], f32)
                nc.vector.reciprocal(di, dn)
                f = pool.tile([p, w], f32)
                nc.vector.tensor_mul(f, n2, rs)
                nc.vector.tensor_mul(f, f, di)
                return f

            # r0 squash on [32, CO]
            sq32 = singles.tile([BL, CO], f32)
            nc.vector.tensor_mul(sq32, s_all, s_all)
            n2_32 = singles.tile([BL, C], f32)
            nc.vector.tensor_reduce(
                n2_32, sq32.rearrange("p (c o) -> p c o", c=C),
                axis=AX.X, op=ALU.add)
            f32t = squash_factor(singles, n2_32, BL, C)
            v0 = singles.tile([BL, CO], f32)
            fb = bass.AP(tensor=f32t.tensor, offset=f32t.offset,
                         ap=[f32t.ap[0], f32t.ap[1], [0, O]])
            nc.vector.tensor_tensor(v0, s_all, fb, op=ALU.mult)
            nc.sync.dma_start(out=vscr_ap, in_=v0)

            for bt in range(NBT):
                # ================= einsum: ihat for this batch tile =========
                ihat = ihp.tile([128, IG * CO], lo)
                for ch in range(IG // XZ_CHUNK):
                    xz_sb = xzp.tile([128, XZ_CHUNK * 128], lo)
                    base = bt * IG + ch * XZ_CHUNK
                    nc.sync.dma_start(
                        out=xz_sb.rearrange("p (t m) -> p t m", t=XZ_CHUNK),
                        in_=xz_ap[base:base + XZ_CHUNK].rearrange(
                            "t p m -> p t m"))
                    for t in range(XZ_CHUNK):
                        ig = ch * XZ_CHUNK + t
                        pih = psum.tile([128, CO], f32)
                        nc.tensor.matmul(pih, xz_sb[:, t * 128:(t + 1) * 128],
                                         w_sb[:, ig * CO:(ig + 1) * CO],
                                         start=True, stop=True)
                        dst = ihat[:, ig * CO:(ig + 1) * CO]
                        if ig % 2 == 0:
                            nc.vector.tensor_copy(dst, pih)
                        else:
                            nc.scalar.copy(dst, pih)

                vsrc = None   # None -> use vscr dram rows for this bt (r0)
                for r in range(NR - 1):
                    # ---- vrep[p=(b,i_sub), co] = v[b, co] ------------------
                    if vsrc is None:
                        vi = bass.AP(tensor=vscr_ap.tensor,
                                     offset=bt * BT * CO,
                                     ap=[[CO, BT], [0, ISUB], [1, CO]])
                    else:
                        vi = bass.AP(tensor=vsrc.tensor, offset=vsrc.offset,
                                     ap=[vsrc.ap[0], [0, ISUB], [1, CO]])
                    nc.gpsimd.dma_start(out=vrep, in_=vi)
                    # ---- bias (+)= sum_o ihat * vrep -----------------------
                    for ch in range(4):
                        g0 = ch * (IG // 4)
                        gn = IG // 4
                        tc_t = tch.tile([128, gn * CO], lo)
                        vb = bass.AP(tensor=vrep.tensor, offset=vrep.offset,
                                     ap=[vrep.ap[0], [0, gn], [1, CO]])
                        nc.vector.tensor_tensor(
                            tc_t, ihat[:, g0 * CO:(g0 + gn) * CO], vb,
                            op=ALU.mult)
                        red_dst = (bias if r == 0 else tmp720)[
                            :, g0 * C:(g0 + gn) * C]
                        nc.vector.tensor_reduce(
                            red_dst,
                            tc_t.rearrange("p (gc o) -> p gc o", o=O),
                            axis=AX.X, op=ALU.add)
                    if r > 0:
                        nc.vector.tensor_add(bias, bias, tmp720)

                    # ---- coup = softmax(bias) over c -----------------------
                    nc.scalar.activation(coup, bias, AF.Exp)
                    nc.vector.tensor_reduce(
                        zsum, coup.rearrange("p (g c) -> p g c", c=C),
                        axis=AX.X, op=ALU.add)
                    rz = sm.tile([128, IG], f32)
                    nc.vector.reciprocal(rz, zsum)
                    rzb = bass.AP(tensor=rz.tensor, offset=rz.offset,
                                  ap=[rz.ap[0], rz.ap[1], [0, C]])
                    nc.vector.tensor_tensor(coup, coup, rzb, op=ALU.mult)

                    # ---- zsc[(b,i),(g,c,b')] = coup[(b,i),(g,c)]*d(b,b') ---
                    zr = zsc.rearrange("p (g c b) -> p g c b", c=C, b=BT)
                    cr = coup.rearrange("p (g c) -> p g c", c=C)
                    cb = bass.AP(tensor=cr.tensor, offset=cr.offset,
                                 ap=[cr.ap[0], cr.ap[1], cr.ap[2], [0, BT]])
                    mr = maskz.rearrange("p (c b) -> p c b", b=BT)
                    mb = bass.AP(tensor=mr.tensor, offset=mr.offset,
                                 ap=[mr.ap[0], [0, IG], mr.ap[1], mr.ap[2]])
                    nc.vector.tensor_tensor(zr, cb, mb, op=ALU.mult)

                    # ---- s = sum_i coup*ihat via PE ------------------------
                    pss = psm.tile([C * BT, CO], f32)
                    for ig in range(IG):
                        nc.tensor.matmul(
                            pss, zsc[:, ig * C * BT:(ig + 1) * C * BT],
                            ihat[:, ig * CO:(ig + 1) * CO],
                            start=(ig == 0), stop=(ig == IG - 1))
                    # masked evacuation: sst[(c',b),(c,o)] = pss * d(c,c')
                    sst = sm.tile([C * BT, CO], f32)
                    nc.vector.tensor_tensor(sst, pss, cmask, op=ALU.mult)
                    # n2 per partition (c',b):  sum over free of sst^2
                    sjunk = sm.tile([C * BT, CO], f32)
                    n2_80 = sm.tile([C * BT, 1], f32)
                    nc.vector.scalar_tensor_tensor(
                        sjunk, sst, 1.0, sst, op0=ALU.mult, op1=ALU.mult,
                        accum_out=n2_80)
                    f80 = squash_factor(sm, n2_80, C * BT, 1)
                    v80 = sm.tile([C * BT, CO], f32)
                    nc.vector.tensor_scalar_mul(v80, sst, f80)
                    # collapse (c',b) -> b with selector matmul
                    v8ps = psm.tile([BT, CO], f32)
                    nc.tensor.matmul(v8ps, sel_sb, v80, start=True, stop=True)
                    v_sb = sm.tile([BT, CO], f32)
                    nc.vector.tensor_copy(v_sb, v8ps)
                    vsrc = v_sb

                nc.sync.dma_start(out=out_ap[bt * BT:(bt + 1) * BT, :],
                                  in_=vsrc)

    nc.compile()
    return nc


def _prep_inputs(x, W):
    """Host-side layout transforms (not part of measured HW time)."""
    x = np.ascontiguousarray(x, dtype=F32)
    W = np.ascontiguousarray(W, dtype=F32)
    # W -> [(i_sub, d), (ig, c, o)]
    wr = np.ascontiguousarray(
        W.reshape(IG, ISUB, C, D, O).transpose(1, 3, 0, 2, 4)
    ).reshape(128, IG * CO)

    # x -> per core [core, bt, b, ig, i_sub, d]
    x8 = x.reshape(NCORES, NBT, BT, IG, ISUB, D)

    # block-diagonal lhsT tiles: xz[core, bt, ig, (i_sub,d), (b,i_sub')]
    xz = np.zeros((NCORES, NBT, IG, ISUB, D, 128), dtype=F32)
    isub = np.arange(ISUB)
    for b in range(BT):
        # advanced indexing pulls the i_sub axis to the front
        xz[:, :, :, isub, :, b * ISUB + isub] = \
            x8[:, :, b].transpose(3, 0, 1, 2, 4)
    xz = xz.reshape(NCORES, NBT * IG, 128, 128)

    # compact xT for r0: [core, (i_sub,d), ig, b]
    xt = np.ascontiguousarray(
        x8.reshape(NCORES, BL, IG, ISUB, D).transpose(0, 3, 4, 2, 1)
    ).reshape(NCORES, 128, IG, BL)

    # constants
    cmask = np.zeros((C * BT, CO), dtype=F32)       # [(c',b), (c,o)]
    for c in range(C):
        cmask[c * BT:(c + 1) * BT, c * O:(c + 1) * O] = 1.0
    # maskz[p=(b,i), (c,b')] = 1 iff b' == b
    maskz = np.zeros((128, C * BT), dtype=F32)      # [(b,i_sub), (c,b')]
    for b in range(BT):
        for c in range(C):
            maskz[b * ISUB:(b + 1) * ISUB, c * BT + b] = 1.0
    sel = np.zeros((C * BT, BT), dtype=F32)         # [(c',b), b']
    for c in range(C):
        for b in range(BT):
            sel[c * BT + b, b] = 1.0

    if USE_BF16:
        from ml_dtypes import bfloat16
        xz = xz.astype(bfloat16)
        xt = xt.astype(bfloat16)
        wr = wr.astype(bfloat16)
        maskz = maskz.astype(bfloat16)
    return xz, xt, wr, cmask, maskz, sel


def kernel(x: np.ndarray, W: np.ndarray) -> np.ndarray:
    from concourse import bass_utils

    if "nc" not in _compiled:
        _compiled["nc"] = _build_program()
    nc = _compiled["nc"]

    xz, xt, wr, cmask, maskz, sel = _prep_inputs(np.asarray(x), np.asarray(W))
    in_maps = [{"xz": xz[c], "xt": xt[c], "w": wr,
                "cmask": cmask, "maskz": maskz, "sel": sel}
               for c in range(NCORES)]
    res = bass_utils.run_bass_kernel_spmd(nc, in_maps, list(range(NCORES)))
    out = np.concatenate([res.results[c]["out"] for c in range(NCORES)], axis=0)
    return out.reshape(B, C, O)



# revision 5
# speedup vs baseline: 1.3276x; 1.3276x over previous
"""CapsuleLayer (dynamic routing) Trainium2 kernel.

Full inputs -> batch-sharded over 8 NeuronCores -> full output.

Math (per sample b):
    ihat[i,c,o] = sum_d x[i,d] * W[i,c,d,o]
    bias = 0
    for r in 0..2:
        coup = softmax(bias, axis=c)
        s[c,o] = sum_i coup[i,c] * ihat[i,c,o]
        v = squash(s)
        if r < 2: bias[i,c] += sum_o ihat[i,c,o] * v[c,o]
    return v

Device layout (per core, 32 local samples, batch-tiles of 8):
    SBUF partition dim p = (b, i_sub): p = b*16 + i_sub, free dim (ig, c, o)
    with ig = i // 16 (72 groups).  ihat tile: [128, 72*10*16] bf16.

    einsum: per (bt, ig) one matmul, lhsT = host-prepared block-diagonal
    x tile [(i_sub,d)=128, (b,i_sub')=128], rhs = W chunk [128, 160].
    PSUM outputs are packed 3 groups per [128,480] bank and evacuated with
    one scalar-engine copy each (cast to bf16) to keep the DVE free.

    routing round r: bias update delta[p,(g,c)] = sum_o ihat*vrep uses a
    2x-mode bf16 multiply plus a pairwise o-halving add tree (16->8->4->
    2->1) instead of the 1x tensor_reduce; softmax over c runs chunked
    (36 groups) so ACT/DVE/PE pipeline; the coupling lhsT
    zsc[p,(g,b',c)] = coup * delta(b,b') is an all-bf16 2x multiply; the
    weighted sum runs on the PE (72 matmuls accumulated in PSUM, out
    partitions (b,c)); squash computes sqrt via exp(0.5*ln(x)) so the
    ACT table set (natural_log_exp_and_others) never switches.  v is
    re-broadcast to [p=(b,i16), (c,o)] by a constant-selector matmul;
    for round 2 the PE accumulates broadcast(v0)+broadcast(v1) since by
    softmax shift-linearity bias2 = ihat.(v0+v1).
"""

import sys

if "/opt/trn_rl_repo" not in sys.path:
    sys.path.insert(0, "/opt/trn_rl_repo")

import numpy as np

B, I, D, C, O = 256, 1152, 8, 10, 16
NCORES = 8
BL = B // NCORES            # 32 local samples per core
NBT, BT = 4, 8              # batch tiles
ISUB = 16                   # i's per group
IG = I // ISUB              # 72 groups
CO = C * O                  # 160
EPS = 1e-7
F32 = np.float32

NCH = 2                     # routing chunks per batch tile
GCH = IG // NCH             # 36 groups per chunk
EVB = 3                     # einsum matmuls batched per PSUM bank

_compiled = {}


def _build_program():
    import concourse.bacc as bacc
    import concourse.tile as tile
    import concourse.mybir as mybir
    import concourse.bass as bass

    f32 = mybir.dt.float32
    bf16 = mybir.dt.bfloat16
    nc = bacc.Bacc("TRN2", target_bir_lowering=False, debug=False,
                   num_devices=NCORES)

    xz_t = nc.dram_tensor("xz", [NBT * IG, 128, 128], bf16, kind="ExternalInput")
    xt_t = nc.dram_tensor("xt", [128, IG, BL], bf16, kind="ExternalInput")
    w_t = nc.dram_tensor("w", [128, IG * CO], bf16, kind="ExternalInput")
    cmask_t = nc.dram_tensor("cmask", [BT * C, CO], f32, kind="ExternalInput")
    mask80_t = nc.dram_tensor("mask80", [128, BT * C], bf16, kind="ExternalInput")
    sel_t = nc.dram_tensor("sel", [BT * C, BT], f32, kind="ExternalInput")
    eall_t = nc.dram_tensor("eall", [BL, NBT * 128], bf16, kind="ExternalInput")
    e8_t = nc.dram_tensor("e8", [BT, 128], bf16, kind="ExternalInput")
    out_t = nc.dram_tensor("out", [BL, CO], f32, kind="ExternalOutput")
    xz_ap, xt_ap, w_ap, out_ap = xz_t.ap(), xt_t.ap(), w_t.ap(), out_t.ap()

    AF = mybir.ActivationFunctionType
    ALU = mybir.AluOpType
    AX = mybir.AxisListType

    def vw(t, off, dims):
        """strided view of a tile: dims = [[stride, count], ...] (free)"""
        return bass.AP(tensor=t.tensor, offset=t.offset + off,
                       ap=[t.ap[0]] + dims)

    with tile.TileContext(nc) as tc:
        from contextlib import ExitStack

        with ExitStack() as ctx:
            singles = ctx.enter_context(tc.tile_pool(name="singles", bufs=1))
            xzp = ctx.enter_context(tc.tile_pool(name="xzp", bufs=2))
            ihp = ctx.enter_context(tc.tile_pool(name="ihp", bufs=2))
            eps_p = ctx.enter_context(tc.tile_pool(name="eps", bufs=3, space="PSUM"))
            sps_p = ctx.enter_context(tc.tile_pool(name="sps", bufs=2, space="PSUM"))
            vps_p = ctx.enter_context(tc.tile_pool(name="vps", bufs=1, space="PSUM"))
            # routing state pools (rotate across rounds / chunks)
            tcp = ctx.enter_context(tc.tile_pool(name="tcp", bufs=2))
            trp = ctx.enter_context(tc.tile_pool(name="trp", bufs=2))
            biasp = ctx.enter_context(tc.tile_pool(name="biasp", bufs=2))
            ep = ctx.enter_context(tc.tile_pool(name="ep", bufs=2))
            coupp = ctx.enter_context(tc.tile_pool(name="coupp", bufs=2))
            zsp = ctx.enter_context(tc.tile_pool(name="zsp", bufs=2))
            zscp = ctx.enter_context(tc.tile_pool(name="zscp", bufs=2))
            vrp = ctx.enter_context(tc.tile_pool(name="vrp", bufs=2))
            sm = ctx.enter_context(tc.tile_pool(name="sm", bufs=2))

            # ---------------- constants / inputs ----------------
            w_sb = singles.tile([128, IG * CO], bf16)
            HW = IG * CO // 2
            nc.sync.dma_start(out=w_sb[:, :HW], in_=w_ap[:, :HW])
            nc.sync.dma_start(out=w_sb[:, HW:], in_=w_ap[:, HW:])
            xt_sb = singles.tile([128, IG * BL], bf16)
            nc.sync.dma_start(out=xt_sb,
                              in_=xt_ap.rearrange("p g b -> p (g b)"))
            cmask = singles.tile([BT * C, CO], f32)
            nc.sync.dma_start(out=cmask, in_=cmask_t.ap())
            mask80 = singles.tile([128, BT * C], bf16)
            nc.sync.dma_start(out=mask80, in_=mask80_t.ap())
            sel_sb = singles.tile([BT * C, BT], f32)
            nc.sync.dma_start(out=sel_sb, in_=sel_t.ap())
            eall = singles.tile([BL, NBT * 128], bf16)
            nc.sync.dma_start(out=eall, in_=eall_t.ap())
            e8 = singles.tile([BT, 128], bf16)
            nc.sync.dma_start(out=e8, in_=e8_t.ap())

            # ---------------- einsum chunk ----------------
            def emit_einsum_chunk(bt, ch, ihat):
                xz_sb = xzp.tile([128, GCH * 128], bf16, name="xz_sb")
                base = bt * IG + ch * GCH
                nc.sync.dma_start(
                    out=xz_sb.rearrange("p (t m) -> p t m", t=GCH),
                    in_=xz_ap[base:base + GCH].rearrange("t p m -> p t m"))
                for t3 in range(GCH // EVB):
                    pb = eps_p.tile([128, EVB * CO], f32, name="pb")
                    for j in range(EVB):
                        t = t3 * EVB + j
                        g = ch * GCH + t
                        nc.tensor.matmul(
                            pb[:, j * CO:(j + 1) * CO],
                            xz_sb[:, t * 128:(t + 1) * 128],
                            w_sb[:, g * CO:(g + 1) * CO],
                            start=(j == 0), stop=(j == EVB - 1),
                            skip_group_check=True)
                    g0 = ch * GCH + t3 * EVB
                    nc.scalar.copy(ihat[:, g0 * CO:(g0 + EVB) * CO], pb)

            # ---------------- squash factor (elementwise [p, w]) ---------
            def squash_factor(n2, p, w):
                """f = n2 / ((1+n2) * sqrt(n2+eps));  sqrt = exp(0.5 ln)"""
                n2e = sm.tile([p, w], f32, name="n2e")
                nc.vector.tensor_scalar_add(n2e, n2, EPS)
                lnv = sm.tile([p, w], f32, name="lnv")
                nc.scalar.activation(lnv, n2e, AF.Ln)
                sq = sm.tile([p, w], f32, name="sq")
                nc.scalar.activation(sq, lnv, AF.Exp, scale=0.5)
                dn = sm.tile([p, w], f32, name="dn")
                nc.scalar.add(dn, n2, 1.0)
                pr = sm.tile([p, w], f32, name="pr")
                nc.vector.tensor_tensor(pr, dn, sq, op=ALU.mult)
                rc = sm.tile([p, w], f32, name="rc")
                nc.vector.reciprocal(rc, pr)
                f = sm.tile([p, w], f32, name="f")
                nc.vector.tensor_tensor(f, n2, rc, op=ALU.mult)
                return f

            # ---------------- einsum bt0 + r0 s/v ----------------
            ihat0 = ihp.tile([128, IG * CO], bf16, name="ihat")
            emit_einsum_chunk(0, 0, ihat0)
            emit_einsum_chunk(0, 1, ihat0)

            ps0 = vps_p.tile([128, CO], f32, name="ps0")
            for kc in range(IG):
                nc.tensor.matmul(ps0[:BL], xt_sb[:, kc * BL:(kc + 1) * BL],
                                 w_sb[:, kc * CO:(kc + 1) * CO],
                                 start=(kc == 0), stop=(kc == IG - 1))
            s_all = singles.tile([BL, CO], f32)
            nc.scalar.mul(s_all, ps0[:BL], 1.0 / C)

            # r0 squash on [32, CO] -> v0 (f32) and v0_bf (bf16)
            sq32 = singles.tile([BL, CO], f32)
            nc.scalar.square(sq32, s_all)
            n2_32 = singles.tile([BL, C], f32)
            nc.vector.tensor_reduce(
                n2_32, sq32.rearrange("p (c o) -> p c o", c=C),
                axis=AX.X, op=ALU.add)
            f0 = squash_factor(n2_32, BL, C)
            v0 = singles.tile([BL, CO], f32)
            nc.vector.tensor_tensor(
                v0.rearrange("p (c o) -> p c o", c=C),
                s_all.rearrange("p (c o) -> p c o", c=C),
                vw(f0, 0, [[1, C], [0, O]]), op=ALU.mult)
            v0_bf = singles.tile([BL, CO], bf16)
            nc.scalar.copy(v0_bf, v0)

            # ---------------- routing ----------------
            ihat_cur = ihat0
            for bt in range(NBT):
                ihat = ihat_cur
                ihat_nxt = None
                if bt + 1 < NBT:
                    ihat_nxt = ihp.tile([128, IG * CO], bf16, name="ihat")

                v_sb_bf = None
                v_sb = None
                for r in range(2):
                    # --- vrep: broadcast v0[bt] (+ v1) to [p=(b,i16), co] --
                    pv = vps_p.tile([128, CO], f32, name="pv")
                    nc.tensor.matmul(pv, eall[:, bt * 128:(bt + 1) * 128],
                                     v0_bf, start=True, stop=(r == 0),
                                     skip_group_check=True)
                    if r == 1:
                        nc.tensor.matmul(pv, e8, v_sb_bf, start=False,
                                         stop=True, skip_group_check=True)
                    vrep = vrp.tile([128, CO], bf16, name="vrep")
                    nc.scalar.copy(vrep, pv)

                    # interleave next-bt einsum chunk into PE/ACT queues
                    if ihat_nxt is not None:
                        emit_einsum_chunk(bt + 1, r, ihat_nxt)

                    bias = biasp.tile([128, IG * C], f32, name="bias")
                    e_bf = ep.tile([128, IG * C], bf16, name="e_bf")
                    coup = coupp.tile([128, IG * C], bf16, name="coup")
                    zs = zsp.tile([128, IG], f32, name="zs")
                    zsc = zscp.tile([128, IG * C * BT], bf16, name="zsc")
                    pss = sps_p.tile([BT * C, CO], f32, name="pss")

                    for chk in range(NCH):
                        gc = GCH * C          # 360 (g,c) pairs per chunk
                        g0 = chk * GCH
                        # --- delta = sum_o ihat * vrep  (mult + o-tree) ---
                        tcm = tcp.tile([128, gc * O], bf16, name="tcm")
                        nc.vector.tensor_tensor(
                            tcm.rearrange("p (g co) -> p g co", g=GCH),
                            vw(ihat, g0 * CO, [[CO, GCH], [1, CO]]),
                            vw(vrep, 0, [[0, GCH], [1, CO]]), op=ALU.mult)
                        t1 = trp.tile([128, gc * 8], bf16, name="t1")
                        nc.vector.tensor_tensor(
                            t1.rearrange("p (z o) -> p z o", o=8),
                            vw(tcm, 0, [[O, gc], [1, 8]]),
                            vw(tcm, 8, [[O, gc], [1, 8]]), op=ALU.add)
                        t2 = trp.tile([128, gc * 4], bf16, name="t2")
                        nc.vector.tensor_tensor(
                            t2.rearrange("p (z o) -> p z o", o=4),
                            vw(t1, 0, [[8, gc], [1, 4]]),
                            vw(t1, 4, [[8, gc], [1, 4]]), op=ALU.add)
                        t3 = trp.tile([128, gc * 2], bf16, name="t3")
                        nc.vector.tensor_tensor(
                            t3.rearrange("p (z o) -> p z o", o=2),
                            vw(t2, 0, [[4, gc], [1, 2]]),
                            vw(t2, 2, [[4, gc], [1, 2]]), op=ALU.add)
                        nc.vector.tensor_tensor(
                            vw(bias, chk * gc, [[1, gc]]),
                            vw(t3, 0, [[2, gc]]),
                            vw(t3, 1, [[2, gc]]), op=ALU.add)

                        # --- softmax over c (per g), bf16 ---
                        nc.scalar.activation(
                            vw(e_bf, chk * gc, [[1, gc]]),
                            vw(bias, chk * gc, [[1, gc]]), AF.Exp)
                        nc.vector.tensor_reduce(
                            vw(zs, chk * GCH, [[1, GCH]]),
                            vw(e_bf, chk * gc, [[C, GCH], [1, C]]),
                            axis=AX.X, op=ALU.add)
                        rz = zsp.tile([128, GCH], f32, name="rz")
                        nc.vector.reciprocal(rz, vw(zs, chk * GCH, [[1, GCH]]))
                        nc.vector.tensor_tensor(
                            vw(coup, chk * gc, [[C, GCH], [1, C]]),
                            vw(e_bf, chk * gc, [[C, GCH], [1, C]]),
                            vw(rz, 0, [[1, GCH], [0, C]]),
                            op=ALU.mult)
                        # --- zsc[p,(g,b',c)] = coup * mask80 (all bf16 2x) -
                        nc.vector.tensor_tensor(
                            vw(zsc, chk * GCH * C * BT,
                               [[C * BT, GCH], [C, BT], [1, C]]),
                            vw(coup, chk * gc, [[C, GCH], [0, BT], [1, C]]),
                            vw(mask80, 0, [[0, GCH], [C, BT], [1, C]]),
                            op=ALU.mult)
                        # --- weighted sum on PE ---
                        for gg in range(GCH):
                            g = g0 + gg
                            nc.tensor.matmul(
                                pss, zsc[:, g * C * BT:(g + 1) * C * BT],
                                ihat[:, g * CO:(g + 1) * CO],
                                start=(g == 0), stop=(g == IG - 1))

                    # --- evacuate s, squash, collapse ---
                    sst = sm.tile([BT * C, CO], f32, name="sst")
                    nc.vector.tensor_tensor(sst, pss, cmask, op=ALU.mult)
                    sjunk = sm.tile([BT * C, CO], f32, name="sjunk")
                    n2_80 = sm.tile([BT * C, 1], f32, name="n2_80")
                    nc.scalar.activation(sjunk, sst, AF.Square,
                                         accum_out=n2_80)
                    f80 = squash_factor(n2_80, BT * C, 1)
                    v80 = sm.tile([BT * C, CO], f32, name="v80")
                    nc.vector.tensor_scalar_mul(v80, sst, f80)
                    v8ps = vps_p.tile([128, CO], f32, name="v8ps")
                    nc.tensor.matmul(v8ps[:BT], sel_sb, v80,
                                     start=True, stop=True)
                    v_sb = sm.tile([BT, CO], f32, name="v_sb")
                    nc.scalar.copy(v_sb, v8ps[:BT])
                    if r == 0:
                        v_sb_bf = sm.tile([BT, CO], bf16, name="v_sb_bf")
                        nc.scalar.copy(v_sb_bf, v8ps[:BT])

                nc.sync.dma_start(out=out_ap[bt * BT:(bt + 1) * BT, :],
                                  in_=v_sb)
                ihat_cur = ihat_nxt

    nc.compile()
    return nc


def _prep_inputs(x, W):
    """Host-side layout transforms (not part of measured HW time)."""
    from ml_dtypes import bfloat16

    x = np.ascontiguousarray(x, dtype=F32)
    W = np.ascontiguousarray(W, dtype=F32)
    # W -> [(i_sub, d), (ig, c, o)]
    wr = np.ascontiguousarray(
        W.reshape(IG, ISUB, C, D, O).transpose(1, 3, 0, 2, 4)
    ).reshape(128, IG * CO)

    # x -> per core [core, bt, b, ig, i_sub, d]
    x8 = x.reshape(NCORES, NBT, BT, IG, ISUB, D)

    # block-diagonal lhsT tiles: xz[core, bt, ig, (i_sub,d), (b,i_sub')]
    xz = np.zeros((NCORES, NBT, IG, ISUB, D, 128), dtype=F32)
    isub = np.arange(ISUB)
    for b in range(BT):
        xz[:, :, :, isub, :, b * ISUB + isub] = \
            x8[:, :, b].transpose(3, 0, 1, 2, 4)
    xz = xz.reshape(NCORES, NBT * IG, 128, 128)

    # compact xT for r0: [core, (i_sub,d), ig, b]
    xt = np.ascontiguousarray(
        x8.reshape(NCORES, BL, IG, ISUB, D).transpose(0, 3, 4, 2, 1)
    ).reshape(NCORES, 128, IG, BL)

    # constants
    # cmask[(b,c), (c',o)] = delta(c, c')
    cmask = np.zeros((BT * C, CO), dtype=F32)
    for b in range(BT):
        for c in range(C):
            cmask[b * C + c, c * O:(c + 1) * O] = 1.0
    # mask80[p=(b,i16), (b',c)] = delta(b, b')
    mask80 = np.zeros((128, BT * C), dtype=F32)
    for b in range(BT):
        mask80[b * ISUB:(b + 1) * ISUB, b * C:(b + 1) * C] = 1.0
    # sel[(b,c), b'] = delta(b, b')
    sel = np.zeros((BT * C, BT), dtype=F32)
    for b in range(BT):
        sel[b * C:(b + 1) * C, b] = 1.0
    # eall[k=0..31, (bt, b, i16)] = delta(k, bt*8 + b)
    eall = np.zeros((BL, NBT * 128), dtype=F32)
    for bt in range(NBT):
        for b in range(BT):
            eall[bt * BT + b,
                 bt * 128 + b * ISUB:bt * 128 + (b + 1) * ISUB] = 1.0
    # e8[k=0..7, (b, i16)] = delta(k, b)
    e8 = np.zeros((BT, 128), dtype=F32)
    for b in range(BT):
        e8[b, b * ISUB:(b + 1) * ISUB] = 1.0

    xz = xz.astype(bfloat16)
    xt = xt.astype(bfloat16)
    wr = wr.astype(bfloat16)
    mask80 = mask80.astype(bfloat16)
    eall = eall.astype(bfloat16)
    e8 = e8.astype(bfloat16)

    return [{"xz": xz[c], "xt": xt[c], "w": wr, "cmask": cmask,
             "mask80": mask80, "sel": sel, "eall": eall, "e8": e8}
            for c in range(NCORES)]


def kernel(x: np.ndarray, W: np.ndarray) -> np.ndarray:
    from concourse import bass_utils

    if "nc" not in _compiled:
        _compiled["nc"] = _build_program()
    nc = _compiled["nc"]

    in_maps = _prep_inputs(np.asarray(x), np.asarray(W))
    res = bass_utils.run_bass_kernel_spmd(nc, in_maps, list(range(NCORES)))
    out = np.concatenate([res.results[c]["out"] for c in range(NCORES)], axis=0)
    return out.reshape(B, C, O)


# revision 8
# speedup vs baseline: 1.4689x; 1.1064x over previous
"""CapsuleLayer (dynamic routing) Trainium2 kernel.

Full inputs -> batch-sharded over 8 NeuronCores -> full output.

Math (per sample b):
    ihat[i,c,o] = sum_d x[i,d] * W[i,c,d,o]
    bias = 0
    for r in 0..2:
        coup = softmax(bias, axis=c)
        s[c,o] = sum_i coup[i,c] * ihat[i,c,o]
        v = squash(s)
        if r < 2: bias[i,c] += sum_o ihat[i,c,o] * v[c,o]
    return v

Device layout (per core, 32 local samples, batch-tiles of 8):
    SBUF partition dim p = (b, i_sub): p = b*16 + i_sub, free dim (ig, c, o)
    with ig = i // 16 (72 groups).  ihat tile: [128, 72*10*16] bf16.

    einsum: per (bt, ig) one matmul, lhsT = host-prepared block-diagonal
    x tile [(i_sub,d)=128, (b,i_sub')=128], rhs = W chunk [128, 160].
    PSUM outputs are packed 3 groups per [128,480] bank and evacuated with
    one scalar-engine copy each (cast to bf16) to keep the DVE free.

    routing round r: bias update delta[p,(g,c)] = sum_o ihat*vrep uses a
    2x-mode bf16 multiply plus a pairwise o-halving add tree (16->8->4->
    2->1) instead of the 1x tensor_reduce; softmax over c runs chunked
    (36 groups) so ACT/DVE/PE pipeline; the coupling lhsT
    zsc[p,(g,b',c)] = coup * delta(b,b') is an all-bf16 2x multiply; the
    weighted sum runs on the PE (72 matmuls accumulated in PSUM, out
    partitions (b,c)); squash computes sqrt via exp(0.5*ln(x)) so the
    ACT table set (natural_log_exp_and_others) never switches.  v is
    re-broadcast to [p=(b,i16), (c,o)] by a constant-selector matmul;
    for round 2 the PE accumulates broadcast(v0)+broadcast(v1) since by
    softmax shift-linearity bias2 = ihat.(v0+v1).
"""

import sys

if "/opt/trn_rl_repo" not in sys.path:
    sys.path.insert(0, "/opt/trn_rl_repo")

import numpy as np

B, I, D, C, O = 256, 1152, 8, 10, 16
NCORES = 8
BL = B // NCORES            # 32 local samples per core
NBT, BT = 4, 8              # batch tiles
ISUB = 16                   # i's per group
IG = I // ISUB              # 72 groups
CO = C * O                  # 160
EPS = 1e-7
F32 = np.float32

NCH = 2                     # routing chunks per batch tile
GCH = IG // NCH             # 36 groups per chunk
EVB = 3                     # einsum matmuls batched per PSUM bank

_compiled = {}


def _build_program():
    import concourse.bacc as bacc
    import concourse.tile as tile
    import concourse.mybir as mybir
    import concourse.bass as bass

    f32 = mybir.dt.float32
    bf16 = mybir.dt.bfloat16
    nc = bacc.Bacc("TRN2", target_bir_lowering=False, debug=False,
                   num_devices=NCORES)

    # Pin the ACT engine to the one table set containing every function we
    # use (Exp, Ln, Copy, Identity, Square) so no per-round table reloads
    # are emitted.  Other sets are blanked (not removed: the set id is the
    # list index in act_info.json).
    from concourse.hw_specs import get_activation_tables
    _tabs = get_activation_tables(nc.m.arch)
    for _k in list(_tabs):
        if _k != "natural_log_exp_and_others":
            _tabs[_k] = set()

    xz_t = nc.dram_tensor("xz", [NBT * IG, 128, 128], bf16, kind="ExternalInput")
    xt_t = nc.dram_tensor("xt", [128, IG, BL], bf16, kind="ExternalInput")
    w_t = nc.dram_tensor("w", [128, IG * CO], bf16, kind="ExternalInput")
    cmask_t = nc.dram_tensor("cmask", [BT * C, CO], f32, kind="ExternalInput")
    mask80_t = nc.dram_tensor("mask80", [128, BT * C], bf16, kind="ExternalInput")
    sel_t = nc.dram_tensor("sel", [BT * C, BT], f32, kind="ExternalInput")
    eall_t = nc.dram_tensor("eall", [BL, NBT * 128], bf16, kind="ExternalInput")
    e8_t = nc.dram_tensor("e8", [BT, 128], bf16, kind="ExternalInput")
    out_t = nc.dram_tensor("out", [BL, CO], f32, kind="ExternalOutput")
    xz_ap, xt_ap, w_ap, out_ap = xz_t.ap(), xt_t.ap(), w_t.ap(), out_t.ap()

    AF = mybir.ActivationFunctionType
    ALU = mybir.AluOpType
    AX = mybir.AxisListType

    def vw(t, off, dims):
        """strided view of a tile: dims = [[stride, count], ...] (free)"""
        return bass.AP(tensor=t.tensor, offset=t.offset + off,
                       ap=[t.ap[0]] + dims)

    with tile.TileContext(nc) as tc:
        from contextlib import ExitStack

        with ExitStack() as ctx:
            singles = ctx.enter_context(tc.tile_pool(name="singles", bufs=1))
            xzp = ctx.enter_context(tc.tile_pool(name="xzp", bufs=2))
            ihp = ctx.enter_context(tc.tile_pool(name="ihp", bufs=2))
            eps_p = ctx.enter_context(tc.tile_pool(name="eps", bufs=3, space="PSUM"))
            sps_p = ctx.enter_context(tc.tile_pool(name="sps", bufs=2, space="PSUM"))
            vps_p = ctx.enter_context(tc.tile_pool(name="vps", bufs=1, space="PSUM"))
            # routing state pools (rotate across rounds / chunks)
            tcp = ctx.enter_context(tc.tile_pool(name="tcp", bufs=2))
            trp = ctx.enter_context(tc.tile_pool(name="trp", bufs=2))
            biasp = ctx.enter_context(tc.tile_pool(name="biasp", bufs=2))
            ep = ctx.enter_context(tc.tile_pool(name="ep", bufs=2))
            coupp = ctx.enter_context(tc.tile_pool(name="coupp", bufs=2))
            zsp = ctx.enter_context(tc.tile_pool(name="zsp", bufs=2))
            zscp = ctx.enter_context(tc.tile_pool(name="zscp", bufs=2))
            vrp = ctx.enter_context(tc.tile_pool(name="vrp", bufs=2))
            sm = ctx.enter_context(tc.tile_pool(name="sm", bufs=2))

            # ---------------- constants / inputs ----------------
            w_sb = singles.tile([128, IG * CO], bf16)
            HW = IG * CO // 2
            nc.sync.dma_start(out=w_sb[:, :HW], in_=w_ap[:, :HW])
            nc.sync.dma_start(out=w_sb[:, HW:], in_=w_ap[:, HW:])
            xt_sb = singles.tile([128, IG * BL], bf16)
            nc.sync.dma_start(out=xt_sb,
                              in_=xt_ap.rearrange("p g b -> p (g b)"))
            cmask = singles.tile([BT * C, CO], f32)
            nc.sync.dma_start(out=cmask, in_=cmask_t.ap())
            mask80 = singles.tile([128, BT * C], bf16)
            nc.sync.dma_start(out=mask80, in_=mask80_t.ap())
            sel_sb = singles.tile([BT * C, BT], f32)
            nc.sync.dma_start(out=sel_sb, in_=sel_t.ap())
            eall = singles.tile([BL, NBT * 128], bf16)
            nc.sync.dma_start(out=eall, in_=eall_t.ap())
            e8 = singles.tile([BT, 128], bf16)
            nc.sync.dma_start(out=e8, in_=e8_t.ap())

            # ---------------- einsum chunk ----------------
            def emit_einsum_chunk(bt, ch, ihat):
                xz_sb = xzp.tile([128, GCH * 128], bf16, name="xz_sb")
                base = bt * IG + ch * GCH
                nc.sync.dma_start(
                    out=xz_sb.rearrange("p (t m) -> p t m", t=GCH),
                    in_=xz_ap[base:base + GCH].rearrange("t p m -> p t m"))
                for t3 in range(GCH // EVB):
                    pb = eps_p.tile([128, EVB * CO], f32, name="pb")
                    for j in range(EVB):
                        t = t3 * EVB + j
                        g = ch * GCH + t
                        nc.tensor.matmul(
                            pb[:, j * CO:(j + 1) * CO],
                            xz_sb[:, t * 128:(t + 1) * 128],
                            w_sb[:, g * CO:(g + 1) * CO],
                            start=(j == 0), stop=(j == EVB - 1),
                            skip_group_check=True)
                    g0 = ch * GCH + t3 * EVB
                    nc.scalar.copy(ihat[:, g0 * CO:(g0 + EVB) * CO], pb)

            # ---------------- squash factor (elementwise [p, w]) ---------
            def squash_factor(n2, p, w):
                """f = n2 / ((1+n2) * sqrt(n2+eps));  sqrt = exp(0.5 ln)"""
                n2e = sm.tile([p, w], f32, name="n2e")
                nc.vector.tensor_scalar_add(n2e, n2, EPS)
                lnv = sm.tile([p, w], f32, name="lnv")
                nc.scalar.activation(lnv, n2e, AF.Ln)
                sq = sm.tile([p, w], f32, name="sq")
                nc.scalar.activation(sq, lnv, AF.Exp, scale=0.5)
                dn = sm.tile([p, w], f32, name="dn")
                nc.scalar.add(dn, n2, 1.0)
                pr = sm.tile([p, w], f32, name="pr")
                nc.vector.tensor_tensor(pr, dn, sq, op=ALU.mult)
                rc = sm.tile([p, w], f32, name="rc")
                nc.vector.reciprocal(rc, pr)
                f = sm.tile([p, w], f32, name="f")
                nc.vector.tensor_tensor(f, n2, rc, op=ALU.mult)
                return f

            # ---------------- einsum bt0 + r0 s/v ----------------
            ihat0 = ihp.tile([128, IG * CO], bf16, name="ihat")
            emit_einsum_chunk(0, 0, ihat0)
            emit_einsum_chunk(0, 1, ihat0)

            ps0 = vps_p.tile([128, CO], f32, name="pv")
            for kc in range(IG):
                nc.tensor.matmul(ps0[:BL], xt_sb[:, kc * BL:(kc + 1) * BL],
                                 w_sb[:, kc * CO:(kc + 1) * CO],
                                 start=(kc == 0), stop=(kc == IG - 1))
            s_all = singles.tile([BL, CO], f32)
            nc.scalar.mul(s_all, ps0[:BL], 1.0 / C)

            # r0 squash on [32, CO] -> v0 (f32) and v0_bf (bf16)
            sq32 = singles.tile([BL, CO], f32)
            nc.scalar.square(sq32, s_all)
            n2_32 = singles.tile([BL, C], f32)
            nc.vector.tensor_reduce(
                n2_32, sq32.rearrange("p (c o) -> p c o", c=C),
                axis=AX.X, op=ALU.add)
            f0 = squash_factor(n2_32, BL, C)
            v0 = singles.tile([BL, CO], f32)
            nc.vector.tensor_tensor(
                v0.rearrange("p (c o) -> p c o", c=C),
                s_all.rearrange("p (c o) -> p c o", c=C),
                vw(f0, 0, [[1, C], [0, O]]), op=ALU.mult)
            v0_bf = singles.tile([BL, CO], bf16)
            nc.scalar.copy(v0_bf, v0)

            # ---------------- routing ----------------
            def emit_delta(ihat, vrep, bias, chk):
                gc = GCH * C              # 360 (g,c) pairs per chunk
                g0 = chk * GCH
                tcm = tcp.tile([128, gc * O], bf16, name="tcm")
                nc.vector.tensor_tensor(
                    tcm.rearrange("p (g co) -> p g co", g=GCH),
                    vw(ihat, g0 * CO, [[CO, GCH], [1, CO]]),
                    vw(vrep, 0, [[0, GCH], [1, CO]]), op=ALU.mult)
                t1 = trp.tile([128, gc * 8], bf16, name="t1")
                nc.vector.tensor_tensor(
                    t1.rearrange("p (z o) -> p z o", o=8),
                    vw(tcm, 0, [[O, gc], [1, 8]]),
                    vw(tcm, 8, [[O, gc], [1, 8]]), op=ALU.add)
                t2 = trp.tile([128, gc * 4], bf16, name="t2")
                nc.vector.tensor_tensor(
                    t2.rearrange("p (z o) -> p z o", o=4),
                    vw(t1, 0, [[8, gc], [1, 4]]),
                    vw(t1, 4, [[8, gc], [1, 4]]), op=ALU.add)
                t3 = trp.tile([128, gc * 2], bf16, name="t3")
                nc.vector.tensor_tensor(
                    t3.rearrange("p (z o) -> p z o", o=2),
                    vw(t2, 0, [[4, gc], [1, 2]]),
                    vw(t2, 2, [[4, gc], [1, 2]]), op=ALU.add)
                nc.vector.tensor_tensor(
                    vw(bias, chk * gc, [[1, gc]]),
                    vw(t3, 0, [[2, gc]]),
                    vw(t3, 1, [[2, gc]]), op=ALU.add)

            def emit_softmax(ihat, bias, e_bf, coup, zs, zsc, pss, chk):
                gc = GCH * C
                g0 = chk * GCH
                nc.scalar.activation(
                    vw(e_bf, chk * gc, [[1, gc]]),
                    vw(bias, chk * gc, [[1, gc]]), AF.Exp)
                nc.vector.tensor_reduce(
                    vw(zs, chk * GCH, [[1, GCH]]),
                    vw(e_bf, chk * gc, [[C, GCH], [1, C]]),
                    axis=AX.X, op=ALU.add)
                rz = zsp.tile([128, GCH], f32, name="rz")
                nc.vector.reciprocal(rz, vw(zs, chk * GCH, [[1, GCH]]))
                nc.vector.tensor_tensor(
                    vw(coup, chk * gc, [[C, GCH], [1, C]]),
                    vw(e_bf, chk * gc, [[C, GCH], [1, C]]),
                    vw(rz, 0, [[1, GCH], [0, C]]),
                    op=ALU.mult)
                nc.vector.tensor_tensor(
                    vw(zsc, chk * GCH * C * BT,
                       [[C * BT, GCH], [C, BT], [1, C]]),
                    vw(coup, chk * gc, [[C, GCH], [0, BT], [1, C]]),
                    vw(mask80, 0, [[0, GCH], [C, BT], [1, C]]),
                    op=ALU.mult)
                for gg in range(GCH):
                    g = g0 + gg
                    nc.tensor.matmul(
                        pss, zsc[:, g * C * BT:(g + 1) * C * BT],
                        ihat[:, g * CO:(g + 1) * CO],
                        start=(g == 0), stop=(g == IG - 1))

            def emit_tail(pss, want_bf):
                """s evac + squash + collapse -> (v_sb f32, v_sb bf16?)"""
                sst = sm.tile([BT * C, CO], f32, name="sst")
                nc.vector.tensor_tensor(sst, pss, cmask, op=ALU.mult)
                sjunk = sm.tile([BT * C, CO], f32, name="sjunk")
                n2_80 = sm.tile([BT * C, 1], f32, name="n2_80")
                nc.scalar.activation(sjunk, sst, AF.Square,
                                     accum_out=n2_80)
                f80 = squash_factor(n2_80, BT * C, 1)
                v80 = sm.tile([BT * C, CO], f32, name="v80")
                nc.vector.tensor_scalar_mul(v80, sst, f80)
                v8ps = vps_p.tile([128, CO], f32, name="v8ps", bufs=2)
                nc.tensor.matmul(v8ps[:BT], sel_sb, v80,
                                 start=True, stop=True)
                v_sb = sm.tile([BT, CO], f32, name="v_sb")
                nc.scalar.copy(v_sb, v8ps[:BT])
                v_bf = None
                if want_bf:
                    v_bf = sm.tile([BT, CO], bf16, name="v_sb_bf")
                    nc.scalar.copy(v_bf, v8ps[:BT])
                return v_sb, v_bf

            ihat_cur = ihat0
            pending = None      # deferred (bt, r1) tail + output DMA
            for bt in range(NBT):
                ihat = ihat_cur
                ihat_nxt = None
                if bt + 1 < NBT:
                    ihat_nxt = ihp.tile([128, IG * CO], bf16, name="ihat")

                v_sb_bf = None
                for r in range(2):
                    # --- vrep: broadcast v0[bt] (+ v1) to [p=(b,i16), co] --
                    pv = vps_p.tile([128, CO], f32, name="pv")
                    nc.tensor.matmul(pv, eall[:, bt * 128:(bt + 1) * 128],
                                     v0_bf, start=True, stop=(r == 0),
                                     skip_group_check=True)
                    if r == 1:
                        nc.tensor.matmul(pv, e8, v_sb_bf, start=False,
                                         stop=True, skip_group_check=True)
                    vrep = vrp.tile([128, CO], bf16, name="vrep")
                    nc.scalar.copy(vrep, pv)

                    # interleave next-bt einsum chunk into PE/ACT queues
                    if ihat_nxt is not None:
                        emit_einsum_chunk(bt + 1, r, ihat_nxt)

                    bias = biasp.tile([128, IG * C], f32, name="bias")
                    e_bf = ep.tile([128, IG * C], bf16, name="e_bf")
                    coup = coupp.tile([128, IG * C], bf16, name="coup")
                    zs = zsp.tile([128, IG], f32, name="zs")
                    zsc = zscp.tile([128, IG * C * BT], bf16, name="zsc")
                    pss = sps_p.tile([BT * C, CO], f32, name="pss")

                    for chk in range(NCH):
                        emit_delta(ihat, vrep, bias, chk)
                    if r == 0 and pending is not None:
                        pending()       # prev bt r1 tail fills this gap
                        pending = None
                    for chk in range(NCH):
                        emit_softmax(ihat, bias, e_bf, coup, zs, zsc,
                                     pss, chk)

                    if r == 0:
                        _, v_sb_bf = emit_tail(pss, True)
                    else:
                        def _mk(bt=bt, pss=pss):
                            def _go():
                                v_sb, _ = emit_tail(pss, False)
                                nc.sync.dma_start(
                                    out=out_ap[bt * BT:(bt + 1) * BT, :],
                                    in_=v_sb)
                            return _go
                        pending = _mk()
                ihat_cur = ihat_nxt
            pending()

    nc.compile()
    return nc


def _prep_inputs(x, W):
    """Host-side layout transforms (not part of measured HW time)."""
    from ml_dtypes import bfloat16

    x = np.ascontiguousarray(x, dtype=F32)
    W = np.ascontiguousarray(W, dtype=F32)
    # W -> [(i_sub, d), (ig, c, o)]
    wr = np.ascontiguousarray(
        W.reshape(IG, ISUB, C, D, O).transpose(1, 3, 0, 2, 4)
    ).reshape(128, IG * CO)

    # x -> per core [core, bt, b, ig, i_sub, d]
    x8 = x.reshape(NCORES, NBT, BT, IG, ISUB, D)

    # block-diagonal lhsT tiles: xz[core, bt, ig, (i_sub,d), (b,i_sub')]
    xz = np.zeros((NCORES, NBT, IG, ISUB, D, 128), dtype=F32)
    isub = np.arange(ISUB)
    for b in range(BT):
        xz[:, :, :, isub, :, b * ISUB + isub] = \
            x8[:, :, b].transpose(3, 0, 1, 2, 4)
    xz = xz.reshape(NCORES, NBT * IG, 128, 128)

    # compact xT for r0: [core, (i_sub,d), ig, b]
    xt = np.ascontiguousarray(
        x8.reshape(NCORES, BL, IG, ISUB, D).transpose(0, 3, 4, 2, 1)
    ).reshape(NCORES, 128, IG, BL)

    # constants
    # cmask[(b,c), (c',o)] = delta(c, c')
    cmask = np.zeros((BT * C, CO), dtype=F32)
    for b in range(BT):
        for c in range(C):
            cmask[b * C + c, c * O:(c + 1) * O] = 1.0
    # mask80[p=(b,i16), (b',c)] = delta(b, b')
    mask80 = np.zeros((128, BT * C), dtype=F32)
    for b in range(BT):
        mask80[b * ISUB:(b + 1) * ISUB, b * C:(b + 1) * C] = 1.0
    # sel[(b,c), b'] = delta(b, b')
    sel = np.zeros((BT * C, BT), dtype=F32)
    for b in range(BT):
        sel[b * C:(b + 1) * C, b] = 1.0
    # eall[k=0..31, (bt, b, i16)] = delta(k, bt*8 + b)
    eall = np.zeros((BL, NBT * 128), dtype=F32)
    for bt in range(NBT):
        for b in range(BT):
            eall[bt * BT + b,
                 bt * 128 + b * ISUB:bt * 128 + (b + 1) * ISUB] = 1.0
    # e8[k=0..7, (b, i16)] = delta(k, b)
    e8 = np.zeros((BT, 128), dtype=F32)
    for b in range(BT):
        e8[b, b * ISUB:(b + 1) * ISUB] = 1.0

    xz = xz.astype(bfloat16)
    xt = xt.astype(bfloat16)
    wr = wr.astype(bfloat16)
    mask80 = mask80.astype(bfloat16)
    eall = eall.astype(bfloat16)
    e8 = e8.astype(bfloat16)

    return [{"xz": xz[c], "xt": xt[c], "w": wr, "cmask": cmask,
             "mask80": mask80, "sel": sel, "eall": eall, "e8": e8}
            for c in range(NCORES)]


def kernel(x: np.ndarray, W: np.ndarray) -> np.ndarray:
    from concourse import bass_utils

    if "nc" not in _compiled:
        _compiled["nc"] = _build_program()
    nc = _compiled["nc"]

    in_maps = _prep_inputs(np.asarray(x), np.asarray(W))
    res = bass_utils.run_bass_kernel_spmd(nc, in_maps, list(range(NCORES)))
    out = np.concatenate([res.results[c]["out"] for c in range(NCORES)], axis=0)
    return out.reshape(B, C, O)


# revision 11
# speedup vs baseline: 1.5624x; 1.0636x over previous
"""CapsuleLayer (dynamic routing) Trainium2 kernel.

Full inputs -> batch-sharded over 8 NeuronCores -> full output.

Math (per sample b):
    ihat[i,c,o] = sum_d x[i,d] * W[i,c,d,o]
    bias = 0
    for r in 0..2:
        coup = softmax(bias, axis=c)
        s[c,o] = sum_i coup[i,c] * ihat[i,c,o]
        v = squash(s)
        if r < 2: bias[i,c] += sum_o ihat[i,c,o] * v[c,o]
    return v

Device layout (per core, 32 local samples, batch-tiles of 8):
    SBUF partition dim p = (b, i_sub): p = b*16 + i_sub, free dim (ig, c, o)
    with ig = i // 16 (72 groups).  ihat tile: [128, 72*10*16] bf16.

    einsum: per (bt, ig) one matmul, lhsT = host-prepared block-diagonal
    x tile [(i_sub,d)=128, (b,i_sub')=128], rhs = W chunk [128, 160].
    PSUM outputs are packed 3 groups per [128,480] bank and evacuated with
    one scalar-engine copy each (cast to bf16) to keep the DVE free.

    routing round r: bias update delta[p,(g,c)] = sum_o ihat*vrep uses a
    2x-mode bf16 multiply plus a pairwise o-halving add tree (16->8->4->
    2->1) instead of the 1x tensor_reduce; softmax over c runs chunked
    (36 groups) so ACT/DVE/PE pipeline; the coupling lhsT
    zsc[p,(g,b',c)] = coup * delta(b,b') is an all-bf16 2x multiply; the
    weighted sum runs on the PE (72 matmuls accumulated in PSUM, out
    partitions (b,c)); squash computes sqrt via exp(0.5*ln(x)) so the
    ACT table set (natural_log_exp_and_others) never switches.  v is
    re-broadcast to [p=(b,i16), (c,o)] by a constant-selector matmul;
    for round 2 the PE accumulates broadcast(v0)+broadcast(v1) since by
    softmax shift-linearity bias2 = ihat.(v0+v1).
"""

import sys

if "/opt/trn_rl_repo" not in sys.path:
    sys.path.insert(0, "/opt/trn_rl_repo")

import numpy as np

B, I, D, C, O = 256, 1152, 8, 10, 16
NCORES = 8
BL = B // NCORES            # 32 local samples per core
NBT, BT = 4, 8              # batch tiles
ISUB = 16                   # i's per group
IG = I // ISUB              # 72 groups
CO = C * O                  # 160
EPS = 1e-7
F32 = np.float32

NCH = 2                     # routing chunks per batch tile
GCH = IG // NCH             # 36 groups per chunk
EVB = 3                     # einsum matmuls batched per PSUM bank

_compiled = {}


def _build_program():
    import concourse.bacc as bacc
    import concourse.tile as tile
    import concourse.mybir as mybir
    import concourse.bass as bass

    f32 = mybir.dt.float32
    bf16 = mybir.dt.bfloat16
    nc = bacc.Bacc("TRN2", target_bir_lowering=False, debug=False,
                   num_devices=NCORES)

    # Pin the ACT engine to the one table set containing every function we
    # use (Exp, Ln, Copy, Identity, Square) so no per-round table reloads
    # are emitted.  Other sets are blanked (not removed: the set id is the
    # list index in act_info.json).
    from concourse.hw_specs import get_activation_tables
    _tabs = get_activation_tables(nc.m.arch)
    for _k in list(_tabs):
        if _k != "natural_log_exp_and_others":
            _tabs[_k] = set()

    # Register an EPS constant AP so ACT-engine ops can use it as a bias
    # (keeps the whole squash chain on the scalar engine, no DVE hop).
    _eps_t = nc.alloc_sbuf_tensor("const-f32-eps", [128, 1], f32)
    nc.gpsimd.memset(_eps_t.ap(), EPS)
    nc.const_aps.aps[(f32, EPS)] = _eps_t.ap()

    xz_t = nc.dram_tensor("xz", [NBT * IG, 128, 128], bf16, kind="ExternalInput")
    xt_t = nc.dram_tensor("xt", [128, IG, BL], bf16, kind="ExternalInput")
    w_t = nc.dram_tensor("w", [128, IG * CO], bf16, kind="ExternalInput")
    cmask_t = nc.dram_tensor("cmask", [BT * C, CO], f32, kind="ExternalInput")
    mask80_t = nc.dram_tensor("mask80", [128, BT * C], bf16, kind="ExternalInput")
    sel_t = nc.dram_tensor("sel", [BT * C, BT], f32, kind="ExternalInput")
    eall_t = nc.dram_tensor("eall", [BL, NBT * 128], bf16, kind="ExternalInput")
    e8_t = nc.dram_tensor("e8", [BT, 128], bf16, kind="ExternalInput")
    out_t = nc.dram_tensor("out", [BL, CO], f32, kind="ExternalOutput")
    xz_ap, xt_ap, w_ap, out_ap = xz_t.ap(), xt_t.ap(), w_t.ap(), out_t.ap()

    AF = mybir.ActivationFunctionType
    ALU = mybir.AluOpType
    AX = mybir.AxisListType

    def vw(t, off, dims):
        """strided view of a tile: dims = [[stride, count], ...] (free)"""
        return bass.AP(tensor=t.tensor, offset=t.offset + off,
                       ap=[t.ap[0]] + dims)

    with tile.TileContext(nc) as tc:
        from contextlib import ExitStack

        with ExitStack() as ctx:
            singles = ctx.enter_context(tc.tile_pool(name="singles", bufs=1))
            xzp = ctx.enter_context(tc.tile_pool(name="xzp", bufs=2))
            ihp = ctx.enter_context(tc.tile_pool(name="ihp", bufs=2))
            eps_p = ctx.enter_context(tc.tile_pool(name="eps", bufs=3, space="PSUM"))
            sps_p = ctx.enter_context(tc.tile_pool(name="sps", bufs=2, space="PSUM"))
            vps_p = ctx.enter_context(tc.tile_pool(name="vps", bufs=1, space="PSUM"))
            # routing state pools (rotate across rounds / chunks)
            tcp = ctx.enter_context(tc.tile_pool(name="tcp", bufs=2))
            trp = ctx.enter_context(tc.tile_pool(name="trp", bufs=2))
            biasp = ctx.enter_context(tc.tile_pool(name="biasp", bufs=2))
            ep = ctx.enter_context(tc.tile_pool(name="ep", bufs=2))
            coupp = ctx.enter_context(tc.tile_pool(name="coupp", bufs=2))
            zsp = ctx.enter_context(tc.tile_pool(name="zsp", bufs=2))
            zscp = ctx.enter_context(tc.tile_pool(name="zscp", bufs=2))
            vrp = ctx.enter_context(tc.tile_pool(name="vrp", bufs=2))
            sm = ctx.enter_context(tc.tile_pool(name="sm", bufs=2))

            # ---------------- constants / inputs ----------------
            w_sb = singles.tile([128, IG * CO], bf16)
            HW = IG * CO // 2
            nc.sync.dma_start(out=w_sb[:, :HW], in_=w_ap[:, :HW])
            nc.sync.dma_start(out=w_sb[:, HW:], in_=w_ap[:, HW:])
            xt_sb = singles.tile([128, IG * BL], bf16)
            nc.sync.dma_start(out=xt_sb,
                              in_=xt_ap.rearrange("p g b -> p (g b)"))
            cmask = singles.tile([BT * C, CO], f32)
            nc.sync.dma_start(out=cmask, in_=cmask_t.ap())
            mask80 = singles.tile([128, BT * C], bf16)
            nc.sync.dma_start(out=mask80, in_=mask80_t.ap())
            sel_sb = singles.tile([BT * C, BT], f32)
            nc.sync.dma_start(out=sel_sb, in_=sel_t.ap())
            eall = singles.tile([BL, NBT * 128], bf16)
            nc.sync.dma_start(out=eall, in_=eall_t.ap())
            e8 = singles.tile([BT, 128], bf16)
            nc.sync.dma_start(out=e8, in_=e8_t.ap())

            # ---------------- einsum chunk ----------------
            def emit_einsum_chunk(bt, ch, ihat):
                xz_sb = xzp.tile([128, GCH * 128], bf16, name="xz_sb")
                base = bt * IG + ch * GCH
                nc.sync.dma_start(
                    out=xz_sb.rearrange("p (t m) -> p t m", t=GCH),
                    in_=xz_ap[base:base + GCH].rearrange("t p m -> p t m"))
                for t3 in range(GCH // EVB):
                    pb = eps_p.tile([128, EVB * CO], f32, name="pb")
                    for j in range(EVB):
                        t = t3 * EVB + j
                        g = ch * GCH + t
                        nc.tensor.matmul(
                            pb[:, j * CO:(j + 1) * CO],
                            xz_sb[:, t * 128:(t + 1) * 128],
                            w_sb[:, g * CO:(g + 1) * CO],
                            start=(j == 0), stop=(j == EVB - 1),
                            skip_group_check=True)
                    g0 = ch * GCH + t3 * EVB
                    nc.scalar.copy(ihat[:, g0 * CO:(g0 + EVB) * CO], pb)

            # ---------------- squash factor (elementwise [p, w]) ---------
            def squash_factor(n2, p, w):
                """f = n2 / ((1+n2) * sqrt(n2+eps));  sqrt = exp(0.5 ln)"""
                lnv = sm.tile([p, w], f32, name="lnv")
                nc.scalar.activation(lnv, n2, AF.Ln, bias=EPS)
                sq = sm.tile([p, w], f32, name="sq")
                nc.scalar.activation(sq, lnv, AF.Exp, scale=0.5)
                dn = sm.tile([p, w], f32, name="dn")
                nc.scalar.add(dn, n2, 1.0)
                pr = sm.tile([p, w], f32, name="pr")
                nc.vector.tensor_tensor(pr, dn, sq, op=ALU.mult)
                rc = sm.tile([p, w], f32, name="rc")
                nc.vector.reciprocal(rc, pr)
                f = sm.tile([p, w], f32, name="f")
                nc.vector.tensor_tensor(f, n2, rc, op=ALU.mult)
                return f

            # ---------------- einsum bt0 + r0 s/v ----------------
            ihat0 = ihp.tile([128, IG * CO], bf16, name="ihat")
            emit_einsum_chunk(0, 0, ihat0)
            emit_einsum_chunk(0, 1, ihat0)

            ps0 = vps_p.tile([128, CO], f32, name="pv")
            for kc in range(IG):
                nc.tensor.matmul(ps0[:BL], xt_sb[:, kc * BL:(kc + 1) * BL],
                                 w_sb[:, kc * CO:(kc + 1) * CO],
                                 start=(kc == 0), stop=(kc == IG - 1))
            s_all = singles.tile([BL, CO], f32)
            nc.scalar.mul(s_all, ps0[:BL], 1.0 / C)

            # r0 squash on [32, CO] -> v0 (f32) and v0_bf (bf16)
            sq32 = singles.tile([BL, CO], f32)
            nc.scalar.square(sq32, s_all)
            n2_32 = singles.tile([BL, C], f32)
            nc.vector.tensor_reduce(
                n2_32, sq32.rearrange("p (c o) -> p c o", c=C),
                axis=AX.X, op=ALU.add)
            f0 = squash_factor(n2_32, BL, C)
            v0 = singles.tile([BL, CO], f32)
            nc.vector.tensor_tensor(
                v0.rearrange("p (c o) -> p c o", c=C),
                s_all.rearrange("p (c o) -> p c o", c=C),
                vw(f0, 0, [[1, C], [0, O]]), op=ALU.mult)
            v0_bf = singles.tile([BL, CO], bf16)
            nc.scalar.copy(v0_bf, v0)

            # ---------------- routing ----------------
            def emit_delta(ihat, vrep, bias, chk):
                gc = GCH * C              # 360 (g,c) pairs per chunk
                g0 = chk * GCH
                tcm = tcp.tile([128, gc * O], bf16, name="tcm")
                nc.vector.tensor_tensor(
                    tcm.rearrange("p (g co) -> p g co", g=GCH),
                    vw(ihat, g0 * CO, [[CO, GCH], [1, CO]]),
                    vw(vrep, 0, [[0, GCH], [1, CO]]), op=ALU.mult)
                t1 = trp.tile([128, gc * 8], bf16, name="t1")
                nc.vector.tensor_tensor(
                    t1.rearrange("p (z o) -> p z o", o=8),
                    vw(tcm, 0, [[O, gc], [1, 8]]),
                    vw(tcm, 8, [[O, gc], [1, 8]]), op=ALU.add)
                t2 = trp.tile([128, gc * 4], bf16, name="t2")
                nc.vector.tensor_tensor(
                    t2.rearrange("p (z o) -> p z o", o=4),
                    vw(t1, 0, [[8, gc], [1, 4]]),
                    vw(t1, 4, [[8, gc], [1, 4]]), op=ALU.add)
                t3 = trp.tile([128, gc * 2], bf16, name="t3")
                nc.vector.tensor_tensor(
                    t3.rearrange("p (z o) -> p z o", o=2),
                    vw(t2, 0, [[4, gc], [1, 2]]),
                    vw(t2, 2, [[4, gc], [1, 2]]), op=ALU.add)
                nc.vector.tensor_tensor(
                    vw(bias, chk * gc, [[1, gc]]),
                    vw(t3, 0, [[2, gc]]),
                    vw(t3, 1, [[2, gc]]), op=ALU.add)

            def emit_softmax(ihat, bias, e_bf, coup, zs, zsc, pss, chk):
                gc = GCH * C
                g0 = chk * GCH
                nc.scalar.activation(
                    vw(e_bf, chk * gc, [[1, gc]]),
                    vw(bias, chk * gc, [[1, gc]]), AF.Exp)
                nc.vector.tensor_reduce(
                    vw(zs, chk * GCH, [[1, GCH]]),
                    vw(e_bf, chk * gc, [[C, GCH], [1, C]]),
                    axis=AX.X, op=ALU.add)
                rz = zsp.tile([128, GCH], f32, name="rz")
                nc.vector.reciprocal(rz, vw(zs, chk * GCH, [[1, GCH]]))
                nc.vector.tensor_tensor(
                    vw(coup, chk * gc, [[C, GCH], [1, C]]),
                    vw(e_bf, chk * gc, [[C, GCH], [1, C]]),
                    vw(rz, 0, [[1, GCH], [0, C]]),
                    op=ALU.mult)
                nc.vector.tensor_tensor(
                    vw(zsc, chk * GCH * C * BT,
                       [[C * BT, GCH], [C, BT], [1, C]]),
                    vw(coup, chk * gc, [[C, GCH], [0, BT], [1, C]]),
                    vw(mask80, 0, [[0, GCH], [C, BT], [1, C]]),
                    op=ALU.mult)
                for gg in range(GCH):
                    g = g0 + gg
                    nc.tensor.matmul(
                        pss, zsc[:, g * C * BT:(g + 1) * C * BT],
                        ihat[:, g * CO:(g + 1) * CO],
                        start=(g == 0), stop=(g == IG - 1))

            def emit_tail(pss, want_bf):
                """s evac + squash + collapse -> (v_sb f32, v_sb bf16?)"""
                sst = sm.tile([BT * C, CO], f32, name="sst")
                nc.vector.tensor_tensor(sst, pss, cmask, op=ALU.mult)
                sjunk = sm.tile([BT * C, CO], f32, name="sjunk")
                n2_80 = sm.tile([BT * C, 1], f32, name="n2_80")
                nc.scalar.activation(sjunk, sst, AF.Square,
                                     accum_out=n2_80)
                f80 = squash_factor(n2_80, BT * C, 1)
                v80 = sm.tile([BT * C, CO], f32, name="v80")
                nc.vector.tensor_scalar_mul(v80, sst, f80)
                v8ps = vps_p.tile([128, CO], f32, name="v8ps", bufs=2)
                nc.tensor.matmul(v8ps[:BT], sel_sb, v80,
                                 start=True, stop=True)
                v_sb = sm.tile([BT, CO], f32, name="v_sb")
                nc.scalar.copy(v_sb, v8ps[:BT])
                v_bf = None
                if want_bf:
                    v_bf = sm.tile([BT, CO], bf16, name="v_sb_bf")
                    nc.scalar.copy(v_bf, v8ps[:BT])
                return v_sb, v_bf

            def mk_vrep(bt, v_bf):
                """broadcast v0[bt] (+ v1 if v_bf) to [p=(b,i16), (c,o)]"""
                pv = vps_p.tile([128, CO], f32, name="pv")
                nc.tensor.matmul(pv, eall[:, bt * 128:(bt + 1) * 128],
                                 v0_bf, start=True, stop=(v_bf is None),
                                 skip_group_check=True)
                if v_bf is not None:
                    nc.tensor.matmul(pv, e8, v_bf, start=False,
                                     stop=True, skip_group_check=True)
                vrep = vrp.tile([128, CO], bf16, name="vrep")
                nc.scalar.copy(vrep, pv)
                return vrep

            ihat_cur = ihat0
            pending = None      # deferred (bt, r1) tail + output DMA
            nxt_vrep = None     # hoisted r0 vrep/bias of the NEXT bt
            nxt_bias = None     # (its delta chunk 0 is already emitted)
            for bt in range(NBT):
                ihat = ihat_cur
                ihat_nxt = None
                if bt + 1 < NBT:
                    ihat_nxt = ihp.tile([128, IG * CO], bf16, name="ihat")

                v_sb_bf = None
                for r in range(2):
                    if r == 0 and nxt_vrep is not None:
                        vrep, bias = nxt_vrep, nxt_bias
                        nxt_vrep = nxt_bias = None
                        chk0 = 1
                    else:
                        vrep = mk_vrep(bt, v_sb_bf if r == 1 else None)
                        bias = biasp.tile([128, IG * C], f32, name="bias")
                        chk0 = 0

                    # interleave next-bt einsum chunk into PE/ACT queues
                    if ihat_nxt is not None:
                        emit_einsum_chunk(bt + 1, r, ihat_nxt)

                    e_bf = ep.tile([128, IG * C], bf16, name="e_bf")
                    coup = coupp.tile([128, IG * C], bf16, name="coup")
                    zs = zsp.tile([128, IG], f32, name="zs")
                    zsc = zscp.tile([128, IG * C * BT], bf16, name="zsc")
                    pss = sps_p.tile([BT * C, CO], f32, name="pss")

                    for chk in range(chk0, NCH):
                        emit_delta(ihat, vrep, bias, chk)
                    if r == 0 and pending is not None:
                        pending()       # prev bt r1 tail fills this gap
                        pending = None
                    if r == 0 and ihat_nxt is not None:
                        # vrep for next bt ahead of the s-matmuls (PE queue)
                        nxt_vrep = mk_vrep(bt + 1, None)
                    for chk in range(NCH):
                        emit_softmax(ihat, bias, e_bf, coup, zs, zsc,
                                     pss, chk)

                    if r == 0:
                        if ihat_nxt is not None:
                            # next bt's first delta fills the s-mm/tail gap
                            nxt_bias = biasp.tile([128, IG * C], f32,
                                                  name="bias")
                            emit_delta(ihat_nxt, nxt_vrep, nxt_bias, 0)
                        _, v_sb_bf = emit_tail(pss, True)
                    else:
                        def _mk(bt=bt, pss=pss):
                            def _go():
                                v_sb, _ = emit_tail(pss, False)
                                nc.sync.dma_start(
                                    out=out_ap[bt * BT:(bt + 1) * BT, :],
                                    in_=v_sb)
                            return _go
                        pending = _mk()
                ihat_cur = ihat_nxt
            pending()

    nc.compile()
    return nc


def _prep_inputs(x, W):
    """Host-side layout transforms (not part of measured HW time)."""
    from ml_dtypes import bfloat16

    x = np.ascontiguousarray(x, dtype=F32)
    W = np.ascontiguousarray(W, dtype=F32)
    # W -> [(i_sub, d), (ig, c, o)]
    wr = np.ascontiguousarray(
        W.reshape(IG, ISUB, C, D, O).transpose(1, 3, 0, 2, 4)
    ).reshape(128, IG * CO)

    # x -> per core [core, bt, b, ig, i_sub, d]
    x8 = x.reshape(NCORES, NBT, BT, IG, ISUB, D)

    # block-diagonal lhsT tiles: xz[core, bt, ig, (i_sub,d), (b,i_sub')]
    xz = np.zeros((NCORES, NBT, IG, ISUB, D, 128), dtype=F32)
    isub = np.arange(ISUB)
    for b in range(BT):
        xz[:, :, :, isub, :, b * ISUB + isub] = \
            x8[:, :, b].transpose(3, 0, 1, 2, 4)
    xz = xz.reshape(NCORES, NBT * IG, 128, 128)

    # compact xT for r0: [core, (i_sub,d), ig, b]
    xt = np.ascontiguousarray(
        x8.reshape(NCORES, BL, IG, ISUB, D).transpose(0, 3, 4, 2, 1)
    ).reshape(NCORES, 128, IG, BL)

    # constants
    # cmask[(b,c), (c',o)] = delta(c, c')
    cmask = np.zeros((BT * C, CO), dtype=F32)
    for b in range(BT):
        for c in range(C):
            cmask[b * C + c, c * O:(c + 1) * O] = 1.0
    # mask80[p=(b,i16), (b',c)] = delta(b, b')
    mask80 = np.zeros((128, BT * C), dtype=F32)
    for b in range(BT):
        mask80[b * ISUB:(b + 1) * ISUB, b * C:(b + 1) * C] = 1.0
    # sel[(b,c), b'] = delta(b, b')
    sel = np.zeros((BT * C, BT), dtype=F32)
    for b in range(BT):
        sel[b * C:(b + 1) * C, b] = 1.0
    # eall[k=0..31, (bt, b, i16)] = delta(k, bt*8 + b)
    eall = np.zeros((BL, NBT * 128), dtype=F32)
    for bt in range(NBT):
        for b in range(BT):
            eall[bt * BT + b,
                 bt * 128 + b * ISUB:bt * 128 + (b + 1) * ISUB] = 1.0
    # e8[k=0..7, (b, i16)] = delta(k, b)
    e8 = np.zeros((BT, 128), dtype=F32)
    for b in range(BT):
        e8[b, b * ISUB:(b + 1) * ISUB] = 1.0

    xz = xz.astype(bfloat16)
    xt = xt.astype(bfloat16)
    wr = wr.astype(bfloat16)
    mask80 = mask80.astype(bfloat16)
    eall = eall.astype(bfloat16)
    e8 = e8.astype(bfloat16)

    return [{"xz": xz[c], "xt": xt[c], "w": wr, "cmask": cmask,
             "mask80": mask80, "sel": sel, "eall": eall, "e8": e8}
            for c in range(NCORES)]


def kernel(x: np.ndarray, W: np.ndarray) -> np.ndarray:
    from concourse import bass_utils

    if "nc" not in _compiled:
        _compiled["nc"] = _build_program()
    nc = _compiled["nc"]

    in_maps = _prep_inputs(np.asarray(x), np.asarray(W))
    res = bass_utils.run_bass_kernel_spmd(nc, in_maps, list(range(NCORES)))
    out = np.concatenate([res.results[c]["out"] for c in range(NCORES)], axis=0)
    return out.reshape(B, C, O)
